# revision 1
# baseline (speedup 1.0000x reference)
"""DMPNN encoder on 8 Trainium2 NeuronCores (Bass/Tile, SPMD).

Strategy: shard undirected edge pairs across cores (reverse edges stay
local). Each core sorts its edges by dst into a padded layout (392
node-blocks x 384 edge capacity). Message-passing iteration k:
  h_{k-1} = relu(h0 + pW2_k[src] - hW2_{k-2}[rev])
assembled per 128-edge tile from sequential h0, an indirect row gather of
the node table, and a sequential read of the rev-scattered hW2 buffer.
Segment-sum is a one-hot matmul accumulated in PSUM per node block.
Node partials are ReduceScattered; pW2 slices are AllGathered.
x@W1 and x@W3x are hoisted to node space (no per-edge transposes of x).
"""
import sys, os
sys.path.insert(0, "/opt/trn_rl_repo")
import numpy as np

N = 50000
E = 800000
H = 128
NC = 8
ELOC = E // NC            # 100000
NBLK = 392
NPAD = NBLK * 128         # 50176
TPB = 3
CBLK = TPB * 128          # 384
T = NBLK * TPB            # 1176 tiles of 128 edges
EPAD = T * 128            # 150528
NSLICE = NBLK // NC       # 49 blocks per core slice
NG = 512

_prog = None
LAST_EXEC_NS = None


def _build_program():
    global _prog
    if _prog is not None:
        return _prog
    import concourse.bass as bass
    import concourse.mybir as mybir
    import concourse.tile as tile
    from concourse import bacc
    from concourse.masks import make_identity
    from contextlib import ExitStack

    f32 = mybir.dt.float32
    i32 = mybir.dt.int32

    nc = bacc.Bacc("TRN2", target_bir_lowering=False, debug=False, num_devices=NC)

    def inp(name, shape):
        return nc.dram_tensor(name, shape, f32, kind="ExternalInput").ap()

    xT   = inp("xT",   [133, NPAD])
    xsT  = inp("xsT",  [133, NSLICE * 128])
    eaT  = inp("eaT",  [14, EPAD])
    S    = inp("S",    [EPAD, 128])
    GB   = inp("GB",   [NSLICE, 128, NG])
    W1x1 = inp("W1x1", [128, 128])
    W1x2 = inp("W1x2", [5, 128])
    W1e  = inp("W1e",  [14, 128])
    W2   = inp("W2",   [128, 128])
    W3x1 = inp("W3x1", [128, 128])
    W3x2 = inp("W3x2", [5, 128])
    W3v  = inp("W3v",  [128, 128])
    srcT = nc.dram_tensor("srcT", [128, T], i32, kind="ExternalInput").ap()
    revT = nc.dram_tensor("revT", [128, T], i32, kind="ExternalInput").ap()
    outp = nc.dram_tensor("outp", [NG, H], f32, kind="ExternalOutput").ap()

    XW    = nc.dram_tensor("XW",    [NPAD, H], f32).ap()
    XW3   = nc.dram_tensor("XW3",   [NSLICE, 128, H], f32).ap()
    h0d   = nc.dram_tensor("h0d",   [EPAD, H], f32).ap()
    HRA   = nc.dram_tensor("HRA",   [EPAD, H], f32).ap()
    HRB   = nc.dram_tensor("HRB",   [EPAD, H], f32).ap()
    ndin  = nc.dram_tensor("ndin",  [NBLK, 128, H], f32).ap()
    nsl   = nc.dram_tensor("nsl",   [NSLICE, 128, H], f32).ap()
    pw2s  = nc.dram_tensor("pw2s",  [NSLICE * 128, H], f32).ap()
    pw2f  = nc.dram_tensor("pw2f",  [NPAD, H], f32, addr_space="Shared").ap()

    groups = [list(range(NC))]

    with tile.TileContext(nc) as tc, ExitStack() as ctx:
        consts = ctx.enter_context(tc.tile_pool(name="consts", bufs=1))
        sb = ctx.enter_context(tc.tile_pool(name="sb", bufs=3))
        ps_main = ctx.enter_context(tc.tile_pool(name="ps_main", bufs=2, space="PSUM"))
        ps = ps_main

        ident = consts.tile([128, 128], f32)
        make_identity(nc, ident[:])

        def const_tile(src_ap, shape, cname):
            t_ = consts.tile(shape, f32, name=cname, tag=cname)
            nc.sync.dma_start(out=t_[:], in_=src_ap[:])
            return t_

        w1x1 = const_tile(W1x1, [128, 128], "w1x1")
        w1x2 = const_tile(W1x2, [5, 128], "w1x2")
        w1e  = const_tile(W1e,  [14, 128], "w1e")
        w2   = const_tile(W2,   [128, 128], "w2")
        w3x1 = const_tile(W3x1, [128, 128], "w3x1")
        w3x2 = const_tile(W3x2, [5, 128], "w3x2")
        w3v  = const_tile(W3v,  [128, 128], "w3v")
        sidx = consts.tile([128, T], i32)
        nc.sync.dma_start(out=sidx[:], in_=srcT[:])
        ridx = consts.tile([128, T], i32)
        nc.sync.dma_start(out=ridx[:], in_=revT[:])

        # ---- PRE: XW = x @ W1x (all blocks), XW3 = x_slice @ W3x (own slice)
        for b in range(NBLK):
            cols = slice(b * 128, (b + 1) * 128)
            xt1 = sb.tile([128, 128], f32, tag="xt1")
            nc.sync.dma_start(out=xt1[:], in_=xT[0:128, cols])
            xt2 = sb.tile([5, 128], f32, tag="xt2")
            nc.sync.dma_start(out=xt2[:], in_=xT[128:133, cols])
            pw = ps.tile([128, 128], f32, tag="psw")
            nc.tensor.matmul(out=pw[:], lhsT=xt1[:], rhs=w1x1[:], start=True, stop=False)
            nc.tensor.matmul(out=pw[:], lhsT=xt2[:], rhs=w1x2[:], start=False, stop=True)
            xwb = sb.tile([128, 128], f32, tag="xwb")
            nc.vector.tensor_copy(out=xwb[:], in_=pw[:])
            nc.scalar.dma_start(out=XW[b * 128:(b + 1) * 128, :], in_=xwb[:])
        for b in range(NSLICE):
            cols = slice(b * 128, (b + 1) * 128)
            xt1 = sb.tile([128, 128], f32, tag="xt1")
            nc.sync.dma_start(out=xt1[:], in_=xsT[0:128, cols])
            xt2 = sb.tile([5, 128], f32, tag="xt2")
            nc.sync.dma_start(out=xt2[:], in_=xsT[128:133, cols])
            pw = ps.tile([128, 128], f32, tag="psw")
            nc.tensor.matmul(out=pw[:], lhsT=xt1[:], rhs=w3x1[:], start=True, stop=False)
            nc.tensor.matmul(out=pw[:], lhsT=xt2[:], rhs=w3x2[:], start=False, stop=True)
            xwb = sb.tile([128, 128], f32, tag="xwb")
            nc.vector.tensor_copy(out=xwb[:], in_=pw[:])
            nc.scalar.dma_start(out=XW3[b], in_=xwb[:])

        # ---- edge sweeps
        def sweep(k):
            hr_rd = HRA if k == 2 else HRB
            hr_wr = HRA if k == 1 else HRB
            for b in range(NBLK):
                pnode = ps.tile([128, 128], f32, tag="node")
                for j in range(TPB):
                    t = b * TPB + j
                    rows = slice(t * 128, (t + 1) * 128)
                    if k == 1:
                        g = sb.tile([128, 128], f32, tag="g")
                        nc.gpsimd.indirect_dma_start(
                            out=g[:], out_offset=None, in_=XW[:],
                            in_offset=bass.IndirectOffsetOnAxis(ap=sidx[:, t:t + 1], axis=0))
                        eat = sb.tile([14, 128], f32, tag="eat")
                        nc.sync.dma_start(out=eat[:], in_=eaT[:, rows.start:rows.stop])
                        pe = ps.tile([128, 128], f32, tag="pse")
                        nc.tensor.matmul(out=pe[:], lhsT=eat[:], rhs=w1e[:], start=True, stop=True)
                        t1 = sb.tile([128, 128], f32, tag="t1")
                        nc.vector.tensor_add(out=t1[:], in0=g[:], in1=pe[:])
                        h = sb.tile([128, 128], f32, tag="h")
                        nc.vector.tensor_relu(out=h[:], in_=t1[:])
                        nc.scalar.dma_start(out=h0d[rows, :], in_=h[:])
                    else:
                        g = sb.tile([128, 128], f32, tag="g")
                        nc.gpsimd.indirect_dma_start(
                            out=g[:], out_offset=None, in_=pw2f[:],
                            in_offset=bass.IndirectOffsetOnAxis(ap=sidx[:, t:t + 1], axis=0))
                        h0t = sb.tile([128, 128], f32, tag="h0t")
                        nc.sync.dma_start(out=h0t[:], in_=h0d[rows, :])
                        hrt = sb.tile([128, 128], f32, tag="hrt")
                        nc.sync.dma_start(out=hrt[:], in_=hr_rd[rows, :])
                        t1 = sb.tile([128, 128], f32, tag="t1")
                        nc.vector.tensor_sub(out=t1[:], in0=g[:], in1=hrt[:])
                        t2 = sb.tile([128, 128], f32, tag="t2")
                        nc.vector.tensor_add(out=t2[:], in0=t1[:], in1=h0t[:])
                        h = sb.tile([128, 128], f32, tag="h")
                        nc.vector.tensor_relu(out=h[:], in_=t2[:])
                    st = sb.tile([128, 128], f32, tag="St")
                    nc.sync.dma_start(out=st[:], in_=S[rows, :])
                    nc.tensor.matmul(out=pnode[:], lhsT=st[:], rhs=h[:],
                                     start=(j == 0), stop=(j == TPB - 1))
                    if k < 3:
                        pT = ps.tile([128, 128], f32, tag="psT")
                        nc.tensor.transpose(out=pT[:], in_=h[:], identity=ident[:])
                        hT = sb.tile([128, 128], f32, tag="hT")
                        nc.vector.tensor_copy(out=hT[:], in_=pT[:])
                        pw = ps.tile([128, 128], f32, tag="psw")
                        nc.tensor.matmul(out=pw[:], lhsT=hT[:], rhs=w2[:], start=True, stop=True)
                        hw = sb.tile([128, 128], f32, tag="hw")
                        nc.vector.tensor_copy(out=hw[:], in_=pw[:])
                        nc.gpsimd.indirect_dma_start(
                            out=hr_wr[:],
                            out_offset=bass.IndirectOffsetOnAxis(ap=ridx[:, t:t + 1], axis=0),
                            in_=hw[:], in_offset=None)
                nb = sb.tile([128, 128], f32, tag="nb")
                nc.vector.tensor_copy(out=nb[:], in_=pnode[:])
                nc.scalar.dma_start(out=ndin[b], in_=nb[:])

        def collective(k):
            nc.gpsimd.collective_compute(
                "ReduceScatter", mybir.AluOpType.add, replica_groups=groups,
                ins=[ndin[:]], outs=[nsl[:]])
            if k < 3:
                for b in range(NSLICE):
                    nsb = sb.tile([128, 128], f32, tag="nsb")
                    nc.sync.dma_start(out=nsb[:], in_=nsl[b])
                    pT = ps.tile([128, 128], f32, tag="psT")
                    nc.tensor.transpose(out=pT[:], in_=nsb[:], identity=ident[:])
                    nT = sb.tile([128, 128], f32, tag="hT")
                    nc.vector.tensor_copy(out=nT[:], in_=pT[:])
                    pw = ps.tile([128, 128], f32, tag="psw")
                    nc.tensor.matmul(out=pw[:], lhsT=nT[:], rhs=w2[:], start=True, stop=True)
                    pb = sb.tile([128, 128], f32, tag="hw")
                    nc.vector.tensor_copy(out=pb[:], in_=pw[:])
                    nc.scalar.dma_start(out=pw2s[b * 128:(b + 1) * 128, :], in_=pb[:])
                nc.gpsimd.collective_compute(
                    "AllGather", mybir.AluOpType.bypass, replica_groups=groups,
                    ins=[pw2s[:]], outs=[pw2f[:]])

        sweep(1)
        collective(1)
        sweep(2)
        collective(2)
        sweep(3)
        collective(3)

        # ---- final: node_attr = relu(XW3 + vmsg @ W3v); out = GB^T @ node_attr
        out_acc = consts.tile([128, 4 * 128], f32, name="out_acc")
        nc.vector.memset(out_acc[:], 0.0)
        for b in range(NSLICE):
            vb = sb.tile([128, 128], f32, tag="nsb")
            nc.sync.dma_start(out=vb[:], in_=nsl[b])
            pT = ps.tile([128, 128], f32, tag="psT")
            nc.tensor.transpose(out=pT[:], in_=vb[:], identity=ident[:])
            vT = sb.tile([128, 128], f32, tag="hT")
            nc.vector.tensor_copy(out=vT[:], in_=pT[:])
            pn = ps.tile([128, 128], f32, tag="pse")
            nc.tensor.matmul(out=pn[:], lhsT=vT[:], rhs=w3v[:], start=True, stop=True)
            x3b = sb.tile([128, 128], f32, tag="h0t")
            nc.sync.dma_start(out=x3b[:], in_=XW3[b])
            t1 = sb.tile([128, 128], f32, tag="t1")
            nc.vector.tensor_add(out=t1[:], in0=x3b[:], in1=pn[:])
            na = sb.tile([128, 128], f32, tag="h")
            nc.vector.tensor_relu(out=na[:], in_=t1[:])
            gb = sb.tile([128, NG], f32, tag="gb")
            nc.sync.dma_start(out=gb[:], in_=GB[b])
            for g4 in range(4):
                po = ps.tile([128, 128], f32, tag="psw", name="po")
                nc.tensor.matmul(out=po[:], lhsT=gb[:, g4 * 128:(g4 + 1) * 128],
                                 rhs=na[:], start=True, stop=True)
                gsl = slice(g4 * 128, (g4 + 1) * 128)
                nc.vector.tensor_add(out=out_acc[:, gsl], in0=out_acc[:, gsl], in1=po[:])
        for g4 in range(4):
            nc.scalar.dma_start(out=outp[g4 * 128:(g4 + 1) * 128, :],
                                in_=out_acc[:, g4 * 128:(g4 + 1) * 128])

    nc.compile()
    _prog = nc
    return nc


def _host_layout(x, edge_attr, edge_index, batch):
    src_all = np.asarray(edge_index[0]).astype(np.int64)
    dst_all = np.asarray(edge_index[1]).astype(np.int64)
    batch = np.asarray(batch).astype(np.int64)

    xTfull = np.zeros((133, NPAD), np.float32)
    xTfull[:, :N] = np.asarray(x, np.float32).T

    per_core = []
    for c in range(NC):
        lo = c * ELOC
        src = src_all[lo:lo + ELOC]
        dst = dst_all[lo:lo + ELOC]
        order = np.argsort(dst, kind="stable")
        dsts = dst[order]
        blk = dsts >> 7
        cnt = np.bincount(blk, minlength=NBLK)
        assert cnt.max() <= CBLK, f"block overflow {cnt.max()}"
        start = np.zeros(NBLK, np.int64)
        start[1:] = np.cumsum(cnt)[:-1]
        rank = np.arange(ELOC) - start[blk]
        pos_sorted = blk * CBLK + rank
        posmap = np.empty(ELOC, np.int64)
        posmap[order] = pos_sorted

        src_pad = np.zeros(EPAD, np.int32)
        src_pad[pos_sorted] = src[order].astype(np.int32)
        rev_pad = np.arange(EPAD, dtype=np.int32)
        rev_pad[posmap] = posmap[np.arange(ELOC) ^ 1].astype(np.int32)

        Sc = np.zeros((EPAD, 128), np.float32)
        Sc[pos_sorted, (dsts & 127)] = 1.0

        eaTc = np.zeros((14, EPAD), np.float32)
        eaTc[:, pos_sorted] = np.asarray(edge_attr[lo:lo + ELOC], np.float32)[order].T

        nlo = c * NSLICE * 128
        gb_flat = np.zeros((NSLICE * 128, NG), np.float32)
        nodes = np.arange(nlo, min(nlo + NSLICE * 128, N))
        gb_flat[nodes - nlo, batch[nodes]] = 1.0

        per_core.append(dict(
            eaT=np.ascontiguousarray(eaTc),
            S=Sc,
            srcT=np.ascontiguousarray(src_pad.reshape(T, 128).T),
            revT=np.ascontiguousarray(rev_pad.reshape(T, 128).T),
            GB=np.ascontiguousarray(gb_flat.reshape(NSLICE, 128, NG)),
            xsT=np.ascontiguousarray(xTfull[:, nlo:nlo + NSLICE * 128]),
        ))
    return xTfull, per_core


def kernel(x, edge_attr, W1, W2, W3, edge_index, rev_index, batch):
    global LAST_EXEC_NS
    from concourse.bass_utils import run_bass_kernel_spmd

    x = np.asarray(x, np.float32)
    edge_attr = np.asarray(edge_attr, np.float32)
    W1 = np.asarray(W1, np.float32)
    W2m = np.asarray(W2, np.float32)
    W3 = np.asarray(W3, np.float32)

    nc = _build_program()
    xTfull, per_core = _host_layout(x, edge_attr, edge_index, batch)

    shared = dict(
        xT=xTfull,
        W1x1=np.ascontiguousarray(W1[0:128]),
        W1x2=np.ascontiguousarray(W1[128:133]),
        W1e=np.ascontiguousarray(W1[133:147]),
        W2=W2m,
        W3x1=np.ascontiguousarray(W3[0:128]),
        W3x2=np.ascontiguousarray(W3[128:133]),
        W3v=np.ascontiguousarray(W3[133:261]),
    )
    in_maps = [{**shared, **pc} for pc in per_core]

    trace = os.environ.get("BASS_KERNEL_TRACE", "0") == "1"
    import time as _time
    t0 = _time.time()
    res = run_bass_kernel_spmd(nc, in_maps, list(range(NC)), trace=trace)
    t1 = _time.time()
    LAST_EXEC_NS = res.exec_time_ns
    if LAST_EXEC_NS is None:
        LAST_EXEC_NS = int((t1 - t0) * 1e9)  # wall-clock fallback (incl. upload)

    out = np.zeros((NG, H), np.float32)
    for c in range(NC):
        out += res.results[c]["outp"]
    return out



# revision 7
# speedup vs baseline: 27.5775x; 27.5775x over previous
"""DMPNN encoder on 8 Trainium2 NeuronCores (Bass/Tile, SPMD).

Strategy: shard undirected edge pairs across cores (reverse edges stay
local). Each core sorts its edges by dst into a padded layout (392
node-blocks x 384 edge capacity). All large operands live in bf16 and all
one-hot matrices (segment-sum by dst, graph pooling by batch) are built
on-chip from small index vectors (iota + is_equal), so the host->device
upload is ~8 MB/core instead of ~130 MB/core. x is sharded by node slice;
x@W1x is AllGathered on device. Message iteration k fuses
  h_k = relu(h0 + (nodein_k@W2)[src] - (h_{k-1}@W2)[rev])
into: PSUM := g + h0 (DVE), PSUM += hrevT^T @ (-W2) (PE, via a bf16
DMA-transpose load of the rev-scattered h), relu on the scalar engine.
All phases run inside For_i hardware loops (indirect-DMA index columns are
staged into fixed SBUF tiles per iteration), keeping the program to a few
hundred instructions - NEFF load/jit time dominates otherwise.
"""
import sys, os
sys.path.insert(0, "/opt/trn_rl_repo")
import numpy as np
import ml_dtypes

BF16 = ml_dtypes.bfloat16

N = 50000
E = 800000
H = 128
NC = 8
ELOC = E // NC            # 100000
NBLK = 392
NPAD = NBLK * 128         # 50176
TPB = 3
CBLK = TPB * 128          # 384
T = NBLK * TPB            # 1176 tiles of 128 edges
EPAD = T * 128            # 150528
NSLICE = NBLK // NC       # 49 blocks per core slice
NOWN = NSLICE * 128       # 6272 nodes per core
NG = 512

_prog = None
LAST_EXEC_NS = None


def _build_program():
    global _prog
    if _prog is not None:
        return _prog
    import concourse.bass as bass
    from concourse.bass import ds
    import concourse.mybir as mybir
    import concourse.tile as tile
    from concourse import bacc
    from concourse.masks import make_identity
    from contextlib import ExitStack

    f32 = mybir.dt.float32
    bf16 = mybir.dt.bfloat16
    i32 = mybir.dt.int32
    RELU = mybir.ActivationFunctionType.Relu

    nc = bacc.Bacc("TRN2", target_bir_lowering=False, debug=False, num_devices=NC)

    xsT  = nc.dram_tensor("xsT",  [133, NOWN], bf16, kind="ExternalInput").ap()
    eaT  = nc.dram_tensor("eaT",  [14, EPAD], bf16, kind="ExternalInput").ap()
    srcT = nc.dram_tensor("srcT", [128, T], i32, kind="ExternalInput").ap()
    rscT = nc.dram_tensor("rscT", [128, T], i32, kind="ExternalInput").ap()
    dstl = nc.dram_tensor("dstl", [128, T], f32, kind="ExternalInput").ap()
    gbat = nc.dram_tensor("gbat", [128, NSLICE], f32, kind="ExternalInput").ap()
    W1x1 = nc.dram_tensor("W1x1", [128, 128], bf16, kind="ExternalInput").ap()
    W1x2 = nc.dram_tensor("W1x2", [5, 128], bf16, kind="ExternalInput").ap()
    W1e  = nc.dram_tensor("W1e",  [14, 128], bf16, kind="ExternalInput").ap()
    W2nb = nc.dram_tensor("W2nb", [128, 128], bf16, kind="ExternalInput").ap()
    W2f  = nc.dram_tensor("W2f",  [128, 128], f32, kind="ExternalInput").ap()
    W3x1 = nc.dram_tensor("W3x1", [128, 128], bf16, kind="ExternalInput").ap()
    W3x2 = nc.dram_tensor("W3x2", [5, 128], bf16, kind="ExternalInput").ap()
    W3vf = nc.dram_tensor("W3vf", [128, 128], f32, kind="ExternalInput").ap()
    outp = nc.dram_tensor("outp", [NG, H], f32, kind="ExternalOutput").ap()

    XWs  = nc.dram_tensor("XWs",  [NOWN, H], bf16).ap()
    XWf  = nc.dram_tensor("XWf",  [NPAD, H], bf16, addr_space="Shared").ap()
    X3s  = nc.dram_tensor("X3s",  [NOWN, H], f32).ap()
    h0d  = nc.dram_tensor("h0d",  [EPAD, H], bf16).ap()
    HRA  = nc.dram_tensor("HRA",  [EPAD, H], bf16).ap()
    HRB  = nc.dram_tensor("HRB",  [EPAD, H], bf16).ap()
    ndin = nc.dram_tensor("ndin", [NBLK, 128, H], f32).ap()
    nsl  = nc.dram_tensor("nsl",  [NSLICE, 128, H], f32).ap()
    pw2s = nc.dram_tensor("pw2s", [NOWN, H], bf16).ap()
    pw2f = nc.dram_tensor("pw2f", [NPAD, H], bf16, addr_space="Shared").ap()

    groups = [list(range(NC))]

    with tile.TileContext(nc) as tc, ExitStack() as ctx:
        consts = ctx.enter_context(tc.tile_pool(name="consts", bufs=1))
        sb = ctx.enter_context(tc.tile_pool(name="sb", bufs=3))
        ps = ctx.enter_context(tc.tile_pool(name="ps", bufs=2, space="PSUM"))

        ident = consts.tile([128, 128], f32)
        make_identity(nc, ident[:])

        def const_tile(src_ap, shape, dt, cname):
            t_ = consts.tile(shape, dt, name=cname, tag=cname)
            nc.sync.dma_start(out=t_[:], in_=src_ap[:])
            return t_

        w1x1 = const_tile(W1x1, [128, 128], bf16, "w1x1")
        w1x2 = const_tile(W1x2, [5, 128], bf16, "w1x2")
        w1e  = const_tile(W1e,  [14, 128], bf16, "w1e")
        w2nb = const_tile(W2nb, [128, 128], bf16, "w2nb")
        w2f  = const_tile(W2f,  [128, 128], f32, "w2f")
        w3x1 = const_tile(W3x1, [128, 128], bf16, "w3x1")
        w3x2 = const_tile(W3x2, [5, 128], bf16, "w3x2")
        w3vf = const_tile(W3vf, [128, 128], f32, "w3vf")
        sidx = const_tile(srcT, [128, T], i32, "sidx")
        ridx = const_tile(rscT, [128, T], i32, "ridx")
        dloc = const_tile(dstl, [128, T], f32, "dloc")
        gbt  = const_tile(gbat, [128, NSLICE], f32, "gbt")

        iog = []
        for g4 in range(4):
            io = consts.tile([128, 128], f32, name=f"io{g4}", tag=f"io{g4}")
            nc.gpsimd.iota(io[:], pattern=[[1, 128]], base=g4 * 128,
                           channel_multiplier=0,
                           allow_small_or_imprecise_dtypes=True)
            iog.append(io)
        io0 = iog[0]
        # per-j physical staging tiles for indirect-DMA index columns
        icur = [consts.tile([128, 1], i32, name=f"ic{j}", tag=f"ic{j}")
                for j in range(TPB)]
        rcur = [consts.tile([128, 1], i32, name=f"rc{j}", tag=f"rc{j}")
                for j in range(TPB)]
        out_acc = consts.tile([128, 4 * 128], f32, name="out_acc", tag="out_acc")

        # ---- PRE: XWs = x_own @ W1x (bf16), X3s = x_own @ W3x (f32); AG XWs
        with tc.For_i(0, NSLICE) as b:
            xt1 = sb.tile([128, 128], bf16, tag="xt1")
            nc.sync.dma_start(out=xt1[:], in_=xsT[0:128, ds(b * 128, 128)])
            xt2 = sb.tile([5, 128], bf16, tag="xt2")
            nc.sync.dma_start(out=xt2[:], in_=xsT[128:133, ds(b * 128, 128)])
            pw = ps.tile([128, 128], f32, tag="acc")
            nc.tensor.matmul(out=pw[:], lhsT=xt1[:], rhs=w1x1[:], start=True, stop=False)
            nc.tensor.matmul(out=pw[:], lhsT=xt2[:], rhs=w1x2[:], start=False, stop=True)
            xwb = sb.tile([128, 128], bf16, tag="xwb")
            nc.vector.tensor_copy(out=xwb[:], in_=pw[:])
            nc.scalar.dma_start(out=XWs[ds(b * 128, 128), :], in_=xwb[:])
            pw3 = ps.tile([128, 128], f32, tag="pn")
            nc.tensor.matmul(out=pw3[:], lhsT=xt1[:], rhs=w3x1[:], start=True, stop=False)
            nc.tensor.matmul(out=pw3[:], lhsT=xt2[:], rhs=w3x2[:], start=False, stop=True)
            x3b = sb.tile([128, 128], f32, tag="x3b")
            nc.vector.tensor_copy(out=x3b[:], in_=pw3[:])
            nc.scalar.dma_start(out=X3s[ds(b * 128, 128), :], in_=x3b[:])
        nc.gpsimd.collective_compute(
            "AllGather", mybir.AluOpType.bypass, replica_groups=groups,
            ins=[XWs[:]], outs=[XWf[:]])

        # ---- edge sweeps (For_i over node blocks)
        def sweep(k):
            gath = XWf if k == 1 else pw2f
            hr_rd = HRA if k == 2 else HRB
            hr_wr = HRA if k == 1 else HRB
            with tc.For_i(0, NBLK) as b:
                if k > 1:
                    hrT = sb.tile([128, CBLK], bf16, tag="hrT")
                    nc.sync.dma_start(out=hrT[:],
                                      in_=hr_rd[ds(b * CBLK, CBLK), :],
                                      transpose=True)
                pnode = ps.tile([128, 128], f32, tag="pn")
                for j in range(TPB):
                    nc.vector.tensor_copy(out=icur[j][:],
                                          in_=sidx[:, ds(b * TPB + j, 1)])
                    g = sb.tile([128, 128], bf16, tag="g")
                    nc.gpsimd.indirect_dma_start(
                        out=g[:], out_offset=None, in_=gath[:],
                        in_offset=bass.IndirectOffsetOnAxis(
                            ap=icur[j][:, 0:1], axis=0))
                    acc = ps.tile([128, 128], f32, tag="acc")
                    if k == 1:
                        eat = sb.tile([14, 128], bf16, tag="eat")
                        nc.sync.dma_start(
                            out=eat[:],
                            in_=eaT[:, ds((b * TPB + j) * 128, 128)])
                        nc.vector.tensor_copy(out=acc[:], in_=g[:])
                        nc.tensor.matmul(out=acc[:], lhsT=eat[:], rhs=w1e[:],
                                         start=False, stop=True)
                    else:
                        h0t = sb.tile([128, 128], bf16, tag="h0t")
                        nc.sync.dma_start(
                            out=h0t[:],
                            in_=h0d[ds((b * TPB + j) * 128, 128), :])
                        nc.vector.tensor_add(out=acc[:], in0=g[:], in1=h0t[:])
                        nc.tensor.matmul(out=acc[:],
                                         lhsT=hrT[:, j * 128:(j + 1) * 128],
                                         rhs=w2nb[:], start=False, stop=True)
                    h = sb.tile([128, 128], bf16, tag="h")
                    nc.scalar.activation(out=h[:], in_=acc[:], func=RELU)
                    oh = sb.tile([128, 128], bf16, tag="oh")
                    nc.vector.tensor_tensor(
                        out=oh[:],
                        in0=dloc[:, ds(b * TPB + j, 1)].to_broadcast([128, 128]),
                        in1=io0[:], op=mybir.AluOpType.is_equal)
                    nc.tensor.matmul(out=pnode[:], lhsT=oh[:], rhs=h[:],
                                     start=(j == 0), stop=(j == TPB - 1))
                    if k == 1:
                        nc.scalar.dma_start(
                            out=h0d[ds((b * TPB + j) * 128, 128), :], in_=h[:])
                    if k < 3:
                        nc.vector.tensor_copy(out=rcur[j][:],
                                              in_=ridx[:, ds(b * TPB + j, 1)])
                        nc.gpsimd.indirect_dma_start(
                            out=hr_wr[:],
                            out_offset=bass.IndirectOffsetOnAxis(
                                ap=rcur[j][:, 0:1], axis=0),
                            in_=h[:], in_offset=None)
                nb = sb.tile([128, 128], f32, tag="nb")
                nc.vector.tensor_copy(out=nb[:], in_=pnode[:])
                nc.scalar.dma_start(
                    out=ndin[ds(b, 1)].rearrange("o p h -> p (o h)"), in_=nb[:])

        def node_phase(k):
            nc.gpsimd.collective_compute(
                "ReduceScatter", mybir.AluOpType.add, replica_groups=groups,
                ins=[ndin[:]], outs=[nsl[:]])
            if k == 3:
                return
            with tc.For_i(0, NSLICE) as b:
                nsb = sb.tile([128, 128], f32, tag="nsb")
                nc.sync.dma_start(out=nsb[:],
                                  in_=nsl[ds(b, 1)].rearrange("o p h -> p (o h)"))
                pT = ps.tile([128, 128], f32, tag="acc")
                nc.tensor.transpose(out=pT[:], in_=nsb[:], identity=ident[:])
                nT = sb.tile([128, 128], f32, tag="nT")
                nc.vector.tensor_copy(out=nT[:], in_=pT[:])
                pw = ps.tile([128, 128], f32, tag="pn")
                nc.tensor.matmul(out=pw[:], lhsT=nT[:], rhs=w2f[:],
                                 start=True, stop=True)
                pb = sb.tile([128, 128], bf16, tag="pb")
                nc.vector.tensor_copy(out=pb[:], in_=pw[:])
                nc.scalar.dma_start(out=pw2s[ds(b * 128, 128), :], in_=pb[:])
            nc.gpsimd.collective_compute(
                "AllGather", mybir.AluOpType.bypass, replica_groups=groups,
                ins=[pw2s[:]], outs=[pw2f[:]])

        sweep(1)
        node_phase(1)
        sweep(2)
        node_phase(2)
        sweep(3)
        node_phase(3)

        # ---- POST: node_attr = relu(X3s + vmsg@W3v); out_acc += GB^T @ na
        nc.vector.memset(out_acc[:], 0.0)
        with tc.For_i(0, NSLICE) as b:
            vb = sb.tile([128, 128], f32, tag="nsb")
            nc.sync.dma_start(out=vb[:],
                              in_=nsl[ds(b, 1)].rearrange("o p h -> p (o h)"))
            pT = ps.tile([128, 128], f32, tag="acc")
            nc.tensor.transpose(out=pT[:], in_=vb[:], identity=ident[:])
            vT = sb.tile([128, 128], f32, tag="nT")
            nc.vector.tensor_copy(out=vT[:], in_=pT[:])
            pn = ps.tile([128, 128], f32, tag="pn")
            nc.tensor.matmul(out=pn[:], lhsT=vT[:], rhs=w3vf[:],
                             start=True, stop=True)
            x3b = sb.tile([128, 128], f32, tag="x3b")
            nc.sync.dma_start(out=x3b[:], in_=X3s[ds(b * 128, 128), :])
            t1 = sb.tile([128, 128], f32, tag="t1")
            nc.vector.tensor_add(out=t1[:], in0=x3b[:], in1=pn[:])
            na = sb.tile([128, 128], bf16, tag="na")
            nc.scalar.activation(out=na[:], in_=t1[:], func=RELU)
            for g4 in range(4):
                ohg = sb.tile([128, 128], bf16, tag="ohg")
                nc.vector.tensor_tensor(
                    out=ohg[:], in0=gbt[:, ds(b, 1)].to_broadcast([128, 128]),
                    in1=iog[g4][:], op=mybir.AluOpType.is_equal)
                pg = ps.tile([128, 128], f32, tag="acc")
                nc.tensor.matmul(out=pg[:], lhsT=ohg[:], rhs=na[:],
                                 start=True, stop=True)
                gsl = slice(g4 * 128, (g4 + 1) * 128)
                nc.vector.tensor_add(out=out_acc[:, gsl],
                                     in0=out_acc[:, gsl], in1=pg[:])
        for g4 in range(4):
            nc.scalar.dma_start(out=outp[g4 * 128:(g4 + 1) * 128, :],
                                in_=out_acc[:, g4 * 128:(g4 + 1) * 128])

    nc.compile()
    _prog = nc
    return nc


def _host_layout(x, edge_attr, edge_index, batch):
    src_all = np.asarray(edge_index[0]).astype(np.int64)
    dst_all = np.asarray(edge_index[1]).astype(np.int64)
    batch = np.asarray(batch).astype(np.int64)
    x = np.asarray(x, np.float32)
    edge_attr = np.asarray(edge_attr, np.float32)

    xT = x.T  # [133, N]

    per_core = []
    for c in range(NC):
        lo = c * ELOC
        src = src_all[lo:lo + ELOC]
        dst = dst_all[lo:lo + ELOC]
        order = np.argsort(dst, kind="stable")
        dsts = dst[order]
        blk = dsts >> 7
        cnt = np.bincount(blk, minlength=NBLK)
        assert cnt.max() <= CBLK, f"block overflow {cnt.max()}"
        start = np.zeros(NBLK, np.int64)
        start[1:] = np.cumsum(cnt)[:-1]
        rank = np.arange(ELOC) - start[blk]
        pos_sorted = blk * CBLK + rank
        posmap = np.empty(ELOC, np.int64)
        posmap[order] = pos_sorted

        src_pad = np.zeros(EPAD, np.int32)
        src_pad[pos_sorted] = src[order].astype(np.int32)
        rsc_pad = np.arange(EPAD, dtype=np.int32)
        rsc_pad[pos_sorted] = posmap[order ^ 1].astype(np.int32)
        dst_pad = np.full(EPAD, 300.0, np.float32)
        dst_pad[pos_sorted] = (dsts & 127).astype(np.float32)

        eaTc = np.zeros((14, EPAD), BF16)
        eaTc[:, pos_sorted] = edge_attr[lo:lo + ELOC][order].T.astype(BF16)

        nlo = c * NOWN
        xs = np.zeros((133, NOWN), BF16)
        hi = min(nlo + NOWN, N)
        xs[:, :hi - nlo] = xT[:, nlo:hi].astype(BF16)

        gb = np.full(NOWN, 600.0, np.float32)
        gb[:hi - nlo] = batch[nlo:hi].astype(np.float32)

        per_core.append(dict(
            xsT=np.ascontiguousarray(xs),
            eaT=np.ascontiguousarray(eaTc),
            srcT=np.ascontiguousarray(src_pad.reshape(T, 128).T),
            rscT=np.ascontiguousarray(rsc_pad.reshape(T, 128).T),
            dstl=np.ascontiguousarray(dst_pad.reshape(T, 128).T),
            gbat=np.ascontiguousarray(gb.reshape(NSLICE, 128).T),
        ))
    return per_core


def kernel(x, edge_attr, W1, W2, W3, edge_index, rev_index, batch):
    global LAST_EXEC_NS
    from concourse.bass_utils import run_bass_kernel_spmd

    W1 = np.asarray(W1, np.float32)
    W2m = np.asarray(W2, np.float32)
    W3 = np.asarray(W3, np.float32)

    nc = _build_program()
    per_core = _host_layout(x, edge_attr, edge_index, batch)

    shared = dict(
        W1x1=np.ascontiguousarray(W1[0:128]).astype(BF16),
        W1x2=np.ascontiguousarray(W1[128:133]).astype(BF16),
        W1e=np.ascontiguousarray(W1[133:147]).astype(BF16),
        W2nb=np.ascontiguousarray(-W2m).astype(BF16),
        W2f=W2m,
        W3x1=np.ascontiguousarray(W3[0:128]).astype(BF16),
        W3x2=np.ascontiguousarray(W3[128:133]).astype(BF16),
        W3vf=np.ascontiguousarray(W3[133:261]),
    )
    in_maps = [{**shared, **pc} for pc in per_core]

    trace = os.environ.get("BASS_KERNEL_TRACE", "0") == "1"
    import time as _time
    t0 = _time.time()
    res = run_bass_kernel_spmd(nc, in_maps, list(range(NC)), trace=trace)
    t1 = _time.time()
    LAST_EXEC_NS = res.exec_time_ns
    if LAST_EXEC_NS is None:
        LAST_EXEC_NS = int((t1 - t0) * 1e9)  # wall-clock fallback (incl. upload)

    out = np.zeros((NG, H), np.float32)
    for c in range(NC):
        out += res.results[c]["outp"].astype(np.float32)
    return out
